# revision 28
# baseline (speedup 1.0000x reference)
"""Trainium2 Bass kernel for nn_MultiHeadAttention (B=8, S=1024, HID=1024, NH=16).

Data-parallel over batch across 8 NeuronCores (1 batch element/core).
Host prep: weights pre-transposed (WqT pre-scaled by 8, bq by 8), activations
pre-transposed; Q/K path fp32r, V/O path bf16.

Per-core pipeline (single Tile program):
  1. qT/kT = WT @ XT as fp32r matmuls (1 cyc/row); bias added via ACT
     Identity (per-partition AP bias) during PSUM->SBUF copy; output f32r.
  2. v natural [seq,feat] via bf16 matmuls + K=1 bias row; masked by K_mask
     and augmented with [K_mask, 1] columns -> VM bf16.
  3. Attention, qc-half outer, head inner, ctx deferred one head (software
     pipeline): scores psum [128,1024] via 2 fp32r matmuls (K=64); DVE
     rowmax (negated); ACT exp(s-max) -> e bf16 SBUF; one DMA-xbar transpose
     per (h,qc) -> ET[kp, qb4, kb, q].
  4. ctx^T psum [66,512] = sum_kb VM' @ ET; rows 64/65 = U = sum(e*Km),
     D = sum(e); PSUM->SBUF copies on ACT.
  5. Per qc-half: scale = Q_mask/(U + S*1e-8*D) batched; ctx *= scale;
     out = ctx@WoT+bo for that half (overlaps other half's attention).
"""

import numpy as np
import ml_dtypes

import concourse.bass as bass
import concourse.tile as tile
from concourse import bacc, mybir
from concourse.bass_utils import run_bass_kernel_spmd

F32 = mybir.dt.float32
F32R = mybir.dt.float32r
BF16 = mybir.dt.bfloat16
AF = mybir.ActivationFunctionType
ALU = mybir.AluOpType
AX = mybir.AxisListType

B, S, HID, NH, DH = 8, 1024, 1024, 16, 64
P = 128
NP = HID // P        # 8 hid partition-blocks
NSB = S // P         # 8 seq blocks
NPAIR = NH // 2
CREN = float(S) * 1e-8

_CACHE = {}


def _build():
    nc = bacc.Bacc("TRN2", target_bir_lowering=False, debug=False, num_devices=B)

    XqT = nc.dram_tensor("XqT", [HID, S], F32R, kind="ExternalInput").ap()
    XkT = nc.dram_tensor("XkT", [HID, S], F32R, kind="ExternalInput").ap()
    XvT = nc.dram_tensor("XvT", [HID, S], BF16, kind="ExternalInput").ap()
    Qms = nc.dram_tensor("Qms", [48, 512], BF16, kind="ExternalInput").ap()
    Km = nc.dram_tensor("Km", [S], F32, kind="ExternalInput").ap()
    WqT = nc.dram_tensor("WqT", [HID, HID], F32R, kind="ExternalInput").ap()
    WkT = nc.dram_tensor("WkT", [HID, HID], F32R, kind="ExternalInput").ap()
    WvT = nc.dram_tensor("WvT", [HID, HID], BF16, kind="ExternalInput").ap()
    WoT = nc.dram_tensor("WoT", [HID, HID], BF16, kind="ExternalInput").ap()
    bq8 = nc.dram_tensor("bq8", [HID], F32, kind="ExternalInput").ap()
    bkv = nc.dram_tensor("bk", [HID], F32, kind="ExternalInput").ap()
    bvb = nc.dram_tensor("bvb", [HID], BF16, kind="ExternalInput").ap()
    bob = nc.dram_tensor("bob", [HID], BF16, kind="ExternalInput").ap()
    out = nc.dram_tensor("out", [S, HID], F32, kind="ExternalOutput").ap()
    scl_dram = nc.dram_tensor("scl_scratch", [32, 512], F32).ap()

    with tile.TileContext(nc) as tc:
        with (
            tc.tile_pool(name="consts", bufs=1) as consts,
            tc.tile_pool(name="xh", bufs=2) as xh,          # X^T halves
            tc.tile_pool(name="wch", bufs=2) as wch,        # Wq/Wk^T ob-chunks
            tc.tile_pool(name="wbf", bufs=1) as wbf,        # WvT -> WoT bf16
            tc.tile_pool(name="qk", bufs=1) as qkp,
            tc.tile_pool(name="vm", bufs=1) as vmp,
            tc.tile_pool(name="et", bufs=3) as etp,
            tc.tile_pool(name="epool", bufs=4) as epool,
            tc.tile_pool(name="ctx", bufs=1) as ctxp,
            tc.tile_pool(name="smalls", bufs=4) as smalls,
            tc.tile_pool(name="ud", bufs=1) as udp,
            tc.tile_pool(name="pf512", bufs=2) as pf512,
            tc.tile_pool(name="ps_big", bufs=3, space="PSUM") as ps_big,
            tc.tile_pool(name="ps_mid", bufs=2, space="PSUM") as ps_mid,
        ):
            # --- prefetch first projection operands before anything else ---
            def load_xhalf(x_dram, dtype, sc):
                xt = xh.tile([P, NP, 512], dtype, tag="xh")
                nc.sync.dma_start(
                    xt[:],
                    x_dram.rearrange("(m p) s -> p m s", p=P)
                    [:, :, sc * 512:(sc + 1) * 512])
                return xt

            def load_wchunk(w_dram, ob):
                wc = wch.tile([P, NP, P], F32R, tag="wch")
                nc.sync.dma_start(
                    wc[:],
                    w_dram.rearrange("(m p) f -> p m f", p=P)
                    [:, :, ob * P:(ob + 1) * P])
                return wc

            xq_h0 = load_xhalf(XqT, F32R, 0)
            wq0 = load_wchunk(WqT, 0)
            xq_halves = [xq_h0, load_xhalf(XqT, F32R, 1)]

            # ---------------- constants (on ACT DMA queue) ----------------
            km_pi = consts.tile([P, NSB], F32, name="km_pi")
            nc.gpsimd.dma_start(km_pi[:], Km.rearrange("(o p) -> p o", p=P))
            bq8p = consts.tile([P, NP], F32, name="bq8p")
            nc.gpsimd.dma_start(bq8p[:], bq8.rearrange("(o p) -> p o", p=P))
            bkp = consts.tile([P, NP], F32, name="bkp")
            nc.gpsimd.dma_start(bkp[:], bkv.rearrange("(o p) -> p o", p=P))
            onesb = consts.tile([1, P], BF16, name="onesb")
            nc.vector.memset(onesb[:], 1.0)
            brow = consts.tile([1, HID], BF16, name="brow")
            nc.gpsimd.dma_start(brow[:], bvb[None, :])
            ST = consts.tile([48, S], F32, name="ST")
            qmst = consts.tile([48, 512], BF16, name="qmst")
            nc.gpsimd.dma_start(qmst[:], Qms)

            # ---------------- q/k projections (fp32r) ----------------
            qt = qkp.tile([P, NP, S], F32R, name="qt")
            kt = qkp.tile([P, NP, S], F32R, name="kt")

            def qk_projection(x_dram, w_dram, biasp, dst, xhalves, w0):
                if xhalves is None:
                    xhalves = [load_xhalf(x_dram, F32R, sc) for sc in range(2)]
                for ob in range(NP):
                    wc = w0 if (ob == 0 and w0 is not None) \
                        else load_wchunk(w_dram, ob)
                    for sc in range(2):
                        pool = ps_mid if sc == 0 else ps_big
                        pp = pool.tile([P, 512], F32,
                                       tag="mid" if sc == 0 else "big")
                        for m in range(NP):
                            nc.tensor.matmul(pp[:], wc[:, m, :],
                                             xhalves[sc][:, m, :],
                                             start=(m == 0), stop=(m == NP - 1))
                        nc.scalar.activation(
                            dst[:, ob, sc * 512:(sc + 1) * 512], pp[:],
                            AF.Identity, bias=biasp[:, ob:ob + 1], scale=1.0)

            qk_projection(XqT, WqT, bq8p, qt, xq_halves, wq0)
            qk_projection(XkT, WkT, bkp, kt, None, None)

            # ---------------- v projection (bf16, natural layout) ----------
            VM = vmp.tile([P, NSB, NH, 66], BF16, name="VM")
            wvt = wbf.tile([P, NP, HID], BF16, tag="wbf")
            nc.sync.dma_start(wvt[:], WvT.rearrange("(m p) f -> p m f", p=P))
            for sc in range(2):
                xvt = xh.tile([P, NP, 512], BF16, tag="xh")
                nc.sync.dma_start(
                    xvt[:],
                    XvT.rearrange("(m p) s -> p m s", p=P)
                    [:, :, sc * 512:(sc + 1) * 512])
                for sb2 in range(4):
                    sb = sc * 4 + sb2
                    for oc in range(2):
                        pool = ps_mid if oc == 0 else ps_big
                        pp = pool.tile([P, 512], F32,
                                       tag="mid" if oc == 0 else "big")
                        for m in range(NP):
                            nc.tensor.matmul(
                                pp[:], xvt[:, m, sb2 * P:(sb2 + 1) * P],
                                wvt[:, m, oc * 512:(oc + 1) * 512],
                                start=(m == 0), stop=False)
                        nc.tensor.matmul(pp[:], onesb[:],
                                         brow[:, oc * 512:(oc + 1) * 512],
                                         start=False, stop=True)
                        nc.vector.tensor_scalar_mul(
                            VM[:, sb, oc * 8:(oc + 1) * 8, 0:64],
                            pp[:].rearrange("p (h d) -> p h d", h=8),
                            km_pi[:, sb:sb + 1])
            for sb in range(NSB):
                nc.vector.tensor_copy(
                    VM[:, sb, :, 64:65],
                    km_pi[:, sb:sb + 1, None].to_broadcast([P, NH, 1]))
            nc.vector.memset(VM[:, :, :, 65:66], 1.0)

            # ---------------- attention (software-pipelined) ----------------
            ctxu = ctxp.tile([P, NPAIR, S], BF16, name="ctxu")

            def emit_ctx_mm(h, qc, et_c):
                cp = ps_mid.tile([P, 512], F32, tag="mid")
                for kb in range(NSB):
                    nc.tensor.matmul(cp[0:66, :], VM[:, kb, h, :],
                                     et_c[:, :, kb, :],
                                     start=(kb == 0), stop=(kb == NSB - 1))
                return cp

            def emit_ctx_copy(h, qc, cp):
                j, pb = h // 2, 64 * (h % 2)
                idx = qc * 32 + h
                ud = udp.tile([2, 512], F32, tag="ud")
                nc.scalar.activation(ud[:], cp[64:66, :], AF.Copy)
                nc.gpsimd.dma_start(ST[idx:idx + 1, :], ud[:])
                if pb == 0:
                    nc.vector.tensor_copy(
                        ctxu[0:64, j, qc * 512:(qc + 1) * 512], cp[0:64, :])
                else:
                    sg = pf512.tile([64, 512], BF16, tag="pf512")
                    nc.scalar.activation(sg[:], cp[0:64, :], AF.Copy)
                    nc.gpsimd.dma_start(
                        ctxu[64:128, j, qc * 512:(qc + 1) * 512], sg[:])

            wot = None
            pending = []
            copyq = []
            for qc in range(2):
                for h in range(NH):
                    j, pb = h // 2, 64 * (h % 2)
                    # et_c[kp, qb4, kb, q] = e[qb4-block q, kb*128+kp]
                    et_c = etp.tile([P, 4, NSB, P], BF16, tag="etc")

                    def score_tile(qb4):
                        qb = qc * 4 + qb4
                        sp = ps_big.tile([P, S], F32, tag="big")
                        qs = slice(qb * P, (qb + 1) * P)
                        for kc in range(2):
                            ks = slice(kc * 512, (kc + 1) * 512)
                            nc.tensor.matmul(sp[:, ks], qt[pb:pb + 64, j, qs],
                                             kt[pb:pb + 64, j, ks],
                                             start=True, stop=True)
                        nmax = smalls.tile([P, 1], F32, tag="nmax")
                        nc.vector.tensor_reduce(nmax[:], sp[:], axis=AX.X,
                                                op=ALU.max, negate=True)
                        e_buf = epool.tile([P, S], BF16, tag="e")
                        nc.scalar.activation(e_buf[:], sp[:],
                                             AF.Exp, bias=nmax[:], scale=1.0)
                        nc.sync.dma_start(et_c[:, qb4, :, :], e_buf[:],
                                          transpose=True)

                    score_tile(0)
                    score_tile(1)
                    if len(pending) == 2:
                        ph, pqc, pet = pending.pop(0)
                        cp = emit_ctx_mm(ph, pqc, pet)
                        copyq.append((ph, pqc, cp))
                    score_tile(2)
                    score_tile(3)
                    if len(copyq) == 2:
                        emit_ctx_copy(*copyq.pop(0))
                    pending.append((h, qc, et_c))
                for args in pending:
                    ph, pqc, pet = args
                    cp = emit_ctx_mm(ph, pqc, pet)
                    copyq.append((ph, pqc, cp))
                pending = []
                for args in copyq:
                    emit_ctx_copy(*args)
                copyq = []

                # ---- renorm + output projection for this qc-half ----
                r0 = qc * 32
                r0d = qc * 16
                scl = ST[r0:r0 + 16, 0:512]
                nc.vector.scalar_tensor_tensor(
                    scl, ST[r0:r0 + 16, 512:1024], CREN,
                    scl, ALU.mult, ALU.add)
                nc.vector.reciprocal(scl, scl)
                nc.vector.tensor_tensor(scl, scl, qmst[r0:r0 + 16, :],
                                        ALU.mult)
                nc.sync.dma_start(scl_dram[r0d:r0d + 16, :], scl)

                for j in range(NPAIR):
                    sd = pf512.tile([P, 512], F32, tag="pf512")
                    ia = r0d + 2 * j
                    nc.sync.dma_start(
                        sd[0:64, :],
                        scl_dram[ia:ia + 1, :].to_broadcast([64, 512]))
                    nc.sync.dma_start(
                        sd[64:128, :],
                        scl_dram[ia + 1:ia + 2, :].to_broadcast([64, 512]))
                    nc.gpsimd.tensor_tensor(
                        ctxu[:, j, qc * 512:(qc + 1) * 512],
                        ctxu[:, j, qc * 512:(qc + 1) * 512], sd[:], ALU.mult)

                if wot is None:
                    wot = wbf.tile([P, NP, HID], BF16, tag="wbf")
                    nc.sync.dma_start(wot[:],
                                      WoT.rearrange("(m p) f -> p m f", p=P))
                    nc.gpsimd.dma_start(brow[:], bob[None, :])
                for qb in range(qc * 4, qc * 4 + 4):
                    for oc in range(2):
                        op_ = ps_mid.tile([P, 512], F32, tag="mid")
                        for j in range(NPAIR):
                            nc.tensor.matmul(
                                op_[:], ctxu[:, j, qb * P:(qb + 1) * P],
                                wot[:, j, oc * 512:(oc + 1) * 512],
                                start=(j == 0), stop=False)
                        nc.tensor.matmul(op_[:], onesb[:],
                                         brow[:, oc * 512:(oc + 1) * 512],
                                         start=False, stop=True)
                        ot = pf512.tile([P, 512], F32, tag="pf512")
                        nc.vector.tensor_copy(ot[:], op_[:])
                        nc.sync.dma_start(
                            out[qb * P:(qb + 1) * P, oc * 512:(oc + 1) * 512],
                            ot[:])

    nc.compile()
    return nc


def kernel(Q, K, V, Q_mask, K_mask, Wq, bq, Wk, bk, Wv, bv, Wo, bo):
    if "nc" not in _CACHE:
        _CACHE["nc"] = _build()
    nc = _CACHE["nc"]
    Q = np.asarray(Q, np.float32)
    K = np.asarray(K, np.float32)
    V = np.asarray(V, np.float32)
    bf = ml_dtypes.bfloat16
    shared = {
        "WqT": np.ascontiguousarray((8.0 * np.asarray(Wq, np.float32)).T),
        "WkT": np.ascontiguousarray(np.asarray(Wk, np.float32).T),
        "WvT": np.ascontiguousarray(np.asarray(Wv, np.float32).T.astype(bf)),
        "WoT": np.ascontiguousarray(np.asarray(Wo, np.float32).T.astype(bf)),
        "bq8": np.ascontiguousarray(8.0 * np.asarray(bq, np.float32)),
        "bk": np.ascontiguousarray(np.asarray(bk, np.float32)),
        "bvb": np.ascontiguousarray(np.asarray(bv, np.float32).astype(bf)),
        "bob": np.ascontiguousarray(np.asarray(bo, np.float32).astype(bf)),
    }
    in_maps = []
    for i in range(B):
        m = dict(shared)
        m["XqT"] = np.ascontiguousarray(Q[i].T)
        m["XkT"] = np.ascontiguousarray(K[i].T)
        m["XvT"] = np.ascontiguousarray(V[i].T.astype(bf))
        qm = np.asarray(Q_mask[i], np.float32)
        qms = np.zeros((48, 512), np.float32)
        qms[0:16] = qm[0:512]
        qms[32:48] = qm[512:1024]
        m["Qms"] = qms.astype(bf)
        m["Km"] = np.ascontiguousarray(np.asarray(K_mask[i], np.float32))
        in_maps.append(m)
    res = run_bass_kernel_spmd(nc, in_maps, list(range(B)))
    return np.stack([res.results[i]["out"] for i in range(B)], axis=0)


# revision 29
# speedup vs baseline: 1.1114x; 1.1114x over previous
"""Trainium2 Bass kernel for nn_MultiHeadAttention (B=8, S=1024, HID=1024, NH=16).

Data-parallel over batch across 8 NeuronCores (1 batch element/core).
Host prep: weights pre-transposed (WqT pre-scaled by 8, bq by 8), activations
pre-transposed; Q/K path fp32r, V/O path bf16.

Per-core pipeline (single Tile program):
  1. qT/kT = WT @ XT as fp32r matmuls (1 cyc/row); bias added via ACT
     Identity (per-partition AP bias) during PSUM->SBUF copy; output f32r.
  2. v natural [seq,feat] via bf16 matmuls + K=1 bias row; masked by K_mask
     and augmented with [K_mask, 1] columns -> VM bf16.
  3. Attention, qc-half outer, head inner, ctx deferred one head (software
     pipeline): scores psum [128,1024] via 2 fp32r matmuls (K=64); DVE
     rowmax (negated); ACT exp(s-max) -> e bf16 SBUF; one DMA-xbar transpose
     per (h,qc) -> ET[kp, qb4, kb, q].
  4. ctx^T psum [66,512] = sum_kb VM' @ ET; rows 64/65 = U = sum(e*Km),
     D = sum(e); PSUM->SBUF copies on ACT.
  5. Per qc-half: scale = Q_mask/(U + S*1e-8*D) batched; ctx *= scale;
     out = ctx@WoT+bo for that half (overlaps other half's attention).
"""

import numpy as np
import ml_dtypes

import concourse.bass as bass
import concourse.tile as tile
from concourse import bacc, mybir
from concourse.bass_utils import run_bass_kernel_spmd

F32 = mybir.dt.float32
F32R = mybir.dt.float32r
BF16 = mybir.dt.bfloat16
AF = mybir.ActivationFunctionType
ALU = mybir.AluOpType
AX = mybir.AxisListType

B, S, HID, NH, DH = 8, 1024, 1024, 16, 64
P = 128
NP = HID // P        # 8 hid partition-blocks
NSB = S // P         # 8 seq blocks
NPAIR = NH // 2
CREN = float(S) * 1e-8

_CACHE = {}


def _build():
    nc = bacc.Bacc("TRN2", target_bir_lowering=False, debug=False, num_devices=B)

    XqT = nc.dram_tensor("XqT", [HID, S], F32R, kind="ExternalInput").ap()
    XkT = nc.dram_tensor("XkT", [HID, S], F32R, kind="ExternalInput").ap()
    XvT = nc.dram_tensor("XvT", [HID, S], BF16, kind="ExternalInput").ap()
    Qms = nc.dram_tensor("Qms", [48, 512], BF16, kind="ExternalInput").ap()
    Km = nc.dram_tensor("Km", [S], F32, kind="ExternalInput").ap()
    WqT = nc.dram_tensor("WqT", [HID, HID], F32R, kind="ExternalInput").ap()
    WkT = nc.dram_tensor("WkT", [HID, HID], F32R, kind="ExternalInput").ap()
    WvT = nc.dram_tensor("WvT", [HID, HID], BF16, kind="ExternalInput").ap()
    WoT = nc.dram_tensor("WoT", [HID, HID], BF16, kind="ExternalInput").ap()
    bq8 = nc.dram_tensor("bq8", [HID], F32, kind="ExternalInput").ap()
    bkv = nc.dram_tensor("bk", [HID], F32, kind="ExternalInput").ap()
    bvb = nc.dram_tensor("bvb", [HID], BF16, kind="ExternalInput").ap()
    bob = nc.dram_tensor("bob", [HID], BF16, kind="ExternalInput").ap()
    out = nc.dram_tensor("out", [S, HID], F32, kind="ExternalOutput").ap()
    scl_dram = nc.dram_tensor("scl_scratch", [32, 512], F32).ap()

    with tile.TileContext(nc) as tc:
        with (
            tc.tile_pool(name="consts", bufs=1) as consts,
            tc.tile_pool(name="xh", bufs=2) as xh,          # X^T halves
            tc.tile_pool(name="wch", bufs=2) as wch,        # Wq/Wk^T ob-chunks
            tc.tile_pool(name="wbf", bufs=1) as wbf,        # WvT -> WoT bf16
            tc.tile_pool(name="qk", bufs=1) as qkp,
            tc.tile_pool(name="vm", bufs=1) as vmp,
            tc.tile_pool(name="et", bufs=3) as etp,
            tc.tile_pool(name="epool", bufs=2) as epool,
            tc.tile_pool(name="ctx", bufs=1) as ctxp,
            tc.tile_pool(name="smalls", bufs=4) as smalls,
            tc.tile_pool(name="ud", bufs=1) as udp,
            tc.tile_pool(name="pf512", bufs=2) as pf512,
            tc.tile_pool(name="ps_big", bufs=3, space="PSUM") as ps_big,
            tc.tile_pool(name="ps_mid", bufs=2, space="PSUM") as ps_mid,
        ):
            # --- prefetch first projection operands before anything else ---
            def load_xhalf(x_dram, dtype, sc):
                xt = xh.tile([P, NP, 512], dtype, tag="xh")
                nc.sync.dma_start(
                    xt[:],
                    x_dram.rearrange("(m p) s -> p m s", p=P)
                    [:, :, sc * 512:(sc + 1) * 512])
                return xt

            def load_wchunk(w_dram, ob):
                wc = wch.tile([P, NP, P], F32R, tag="wch")
                nc.sync.dma_start(
                    wc[:],
                    w_dram.rearrange("(m p) f -> p m f", p=P)
                    [:, :, ob * P:(ob + 1) * P])
                return wc

            xq_h0 = load_xhalf(XqT, F32R, 0)
            wq0 = load_wchunk(WqT, 0)
            xq_halves = [xq_h0, load_xhalf(XqT, F32R, 1)]

            # ---------------- constants (on ACT DMA queue) ----------------
            km_pi = consts.tile([P, NSB], F32, name="km_pi")
            nc.gpsimd.dma_start(km_pi[:], Km.rearrange("(o p) -> p o", p=P))
            bq8p = consts.tile([P, NP], F32, name="bq8p")
            nc.gpsimd.dma_start(bq8p[:], bq8.rearrange("(o p) -> p o", p=P))
            bkp = consts.tile([P, NP], F32, name="bkp")
            nc.gpsimd.dma_start(bkp[:], bkv.rearrange("(o p) -> p o", p=P))
            onesb = consts.tile([1, P], BF16, name="onesb")
            nc.vector.memset(onesb[:], 1.0)
            brow = consts.tile([1, HID], BF16, name="brow")
            nc.gpsimd.dma_start(brow[:], bvb[None, :])
            ST = consts.tile([48, S], F32, name="ST")
            qmst = consts.tile([48, 512], BF16, name="qmst")
            nc.gpsimd.dma_start(qmst[:], Qms)

            # ---------------- q/k projections (fp32r) ----------------
            qt = qkp.tile([P, NP, S], F32R, name="qt")
            kt = qkp.tile([P, NP, S], F32R, name="kt")

            def qk_projection(x_dram, w_dram, biasp, dst, xhalves, w0):
                if xhalves is None:
                    xhalves = [load_xhalf(x_dram, F32R, sc) for sc in range(2)]
                for ob in range(NP):
                    wc = w0 if (ob == 0 and w0 is not None) \
                        else load_wchunk(w_dram, ob)
                    for sc in range(2):
                        pool = ps_mid if sc == 0 else ps_big
                        pp = pool.tile([P, 512], F32,
                                       tag="mid" if sc == 0 else "big")
                        for m in range(NP):
                            nc.tensor.matmul(pp[:], wc[:, m, :],
                                             xhalves[sc][:, m, :],
                                             start=(m == 0), stop=(m == NP - 1))
                        nc.scalar.activation(
                            dst[:, ob, sc * 512:(sc + 1) * 512], pp[:],
                            AF.Identity, bias=biasp[:, ob:ob + 1], scale=1.0)

            qk_projection(XqT, WqT, bq8p, qt, xq_halves, wq0)
            qk_projection(XkT, WkT, bkp, kt, None, None)

            # ---------------- v projection (bf16, natural layout) ----------
            VM = vmp.tile([P, NSB, NH, 66], BF16, name="VM")
            wvt = wbf.tile([P, NP, HID], BF16, tag="wbf")
            nc.sync.dma_start(wvt[:], WvT.rearrange("(m p) f -> p m f", p=P))
            for sc in range(2):
                xvt = xh.tile([P, NP, 512], BF16, tag="xh")
                nc.sync.dma_start(
                    xvt[:],
                    XvT.rearrange("(m p) s -> p m s", p=P)
                    [:, :, sc * 512:(sc + 1) * 512])
                for sb2 in range(4):
                    sb = sc * 4 + sb2
                    for oc in range(2):
                        pool = ps_mid if oc == 0 else ps_big
                        pp = pool.tile([P, 512], F32,
                                       tag="mid" if oc == 0 else "big")
                        for m in range(NP):
                            nc.tensor.matmul(
                                pp[:], xvt[:, m, sb2 * P:(sb2 + 1) * P],
                                wvt[:, m, oc * 512:(oc + 1) * 512],
                                start=(m == 0), stop=False)
                        nc.tensor.matmul(pp[:], onesb[:],
                                         brow[:, oc * 512:(oc + 1) * 512],
                                         start=False, stop=True)
                        nc.vector.tensor_scalar_mul(
                            VM[:, sb, oc * 8:(oc + 1) * 8, 0:64],
                            pp[:].rearrange("p (h d) -> p h d", h=8),
                            km_pi[:, sb:sb + 1])
            for sb in range(NSB):
                nc.vector.tensor_copy(
                    VM[:, sb, :, 64:65],
                    km_pi[:, sb:sb + 1, None].to_broadcast([P, NH, 1]))
            nc.vector.memset(VM[:, :, :, 65:66], 1.0)

            # ---------------- attention (software-pipelined) ----------------
            ctxu = ctxp.tile([P, NPAIR, S], BF16, name="ctxu")

            def emit_ctx_mm(h, qc, et_c):
                cp = ps_mid.tile([P, 512], F32, tag="mid")
                for kb in range(NSB):
                    nc.tensor.matmul(cp[0:66, :], VM[:, kb, h, :],
                                     et_c[:, :, kb, :],
                                     start=(kb == 0), stop=(kb == NSB - 1))
                return cp

            def emit_ctx_copy(h, qc, cp):
                j, pb = h // 2, 64 * (h % 2)
                idx = qc * 32 + h
                ud = udp.tile([2, 512], F32, tag="ud")
                nc.scalar.activation(ud[:], cp[64:66, :], AF.Copy)
                nc.gpsimd.dma_start(ST[idx:idx + 1, :], ud[:])
                if pb == 0:
                    nc.vector.tensor_copy(
                        ctxu[0:64, j, qc * 512:(qc + 1) * 512], cp[0:64, :])
                else:
                    sg = pf512.tile([64, 512], BF16, tag="pf512")
                    nc.scalar.activation(sg[:], cp[0:64, :], AF.Copy)
                    nc.gpsimd.dma_start(
                        ctxu[64:128, j, qc * 512:(qc + 1) * 512], sg[:])

            wot = None
            pending = []
            copyq = []
            for qc in range(2):
                for h in range(NH):
                    j, pb = h // 2, 64 * (h % 2)
                    # et_c[kp, qb4, kb, q] = e[qb4-block q, kb*128+kp]
                    et_c = etp.tile([P, 4, NSB, P], BF16, tag="etc")
                    hh = qc * 16 + h
                    if hh % 2 == 0:
                        e_buf = epool.tile([P, 4, S], BF16, tag="e")
                    else:
                        e_buf = xh.tile([P, 4, S], BF16, tag="xh")

                    def score_tile(qb4):
                        qb = qc * 4 + qb4
                        sp = ps_big.tile([P, S], F32, tag="big")
                        qs = slice(qb * P, (qb + 1) * P)
                        for kc in range(2):
                            ks = slice(kc * 512, (kc + 1) * 512)
                            nc.tensor.matmul(sp[:, ks], qt[pb:pb + 64, j, qs],
                                             kt[pb:pb + 64, j, ks],
                                             start=True, stop=True)
                        nmax = smalls.tile([P, 1], F32, tag="nmax")
                        nc.vector.tensor_reduce(nmax[:], sp[:], axis=AX.X,
                                                op=ALU.max, negate=True)
                        nc.scalar.activation(e_buf[:, qb4, :], sp[:],
                                             AF.Exp, bias=nmax[:], scale=1.0)

                    score_tile(0)
                    score_tile(1)
                    if len(pending) == 2:
                        ph, pqc, pet = pending.pop(0)
                        cp = emit_ctx_mm(ph, pqc, pet)
                        copyq.append((ph, pqc, cp))
                    score_tile(2)
                    score_tile(3)
                    nc.sync.dma_start(et_c[:], e_buf[:], transpose=True)
                    if len(copyq) == 2:
                        emit_ctx_copy(*copyq.pop(0))
                    pending.append((h, qc, et_c))
                for args in pending:
                    ph, pqc, pet = args
                    cp = emit_ctx_mm(ph, pqc, pet)
                    copyq.append((ph, pqc, cp))
                pending = []
                for args in copyq:
                    emit_ctx_copy(*args)
                copyq = []

                # ---- renorm + output projection for this qc-half ----
                r0 = qc * 32
                r0d = qc * 16
                scl = ST[r0:r0 + 16, 0:512]
                nc.vector.scalar_tensor_tensor(
                    scl, ST[r0:r0 + 16, 512:1024], CREN,
                    scl, ALU.mult, ALU.add)
                nc.vector.reciprocal(scl, scl)
                nc.vector.tensor_tensor(scl, scl, qmst[r0:r0 + 16, :],
                                        ALU.mult)
                nc.sync.dma_start(scl_dram[r0d:r0d + 16, :], scl)

                for j in range(NPAIR):
                    sd = pf512.tile([P, 512], F32, tag="pf512")
                    ia = r0d + 2 * j
                    nc.sync.dma_start(
                        sd[0:64, :],
                        scl_dram[ia:ia + 1, :].to_broadcast([64, 512]))
                    nc.sync.dma_start(
                        sd[64:128, :],
                        scl_dram[ia + 1:ia + 2, :].to_broadcast([64, 512]))
                    nc.gpsimd.tensor_tensor(
                        ctxu[:, j, qc * 512:(qc + 1) * 512],
                        ctxu[:, j, qc * 512:(qc + 1) * 512], sd[:], ALU.mult)

                if wot is None:
                    wot = wbf.tile([P, NP, HID], BF16, tag="wbf")
                    nc.sync.dma_start(wot[:],
                                      WoT.rearrange("(m p) f -> p m f", p=P))
                    nc.gpsimd.dma_start(brow[:], bob[None, :])
                for qb in range(qc * 4, qc * 4 + 4):
                    for oc in range(2):
                        op_ = ps_mid.tile([P, 512], F32, tag="mid")
                        for j in range(NPAIR):
                            nc.tensor.matmul(
                                op_[:], ctxu[:, j, qb * P:(qb + 1) * P],
                                wot[:, j, oc * 512:(oc + 1) * 512],
                                start=(j == 0), stop=False)
                        nc.tensor.matmul(op_[:], onesb[:],
                                         brow[:, oc * 512:(oc + 1) * 512],
                                         start=False, stop=True)
                        ot = pf512.tile([P, 512], F32, tag="pf512")
                        nc.vector.tensor_copy(ot[:], op_[:])
                        nc.sync.dma_start(
                            out[qb * P:(qb + 1) * P, oc * 512:(oc + 1) * 512],
                            ot[:])

    nc.compile()
    return nc


def kernel(Q, K, V, Q_mask, K_mask, Wq, bq, Wk, bk, Wv, bv, Wo, bo):
    if "nc" not in _CACHE:
        _CACHE["nc"] = _build()
    nc = _CACHE["nc"]
    Q = np.asarray(Q, np.float32)
    K = np.asarray(K, np.float32)
    V = np.asarray(V, np.float32)
    bf = ml_dtypes.bfloat16
    shared = {
        "WqT": np.ascontiguousarray((8.0 * np.asarray(Wq, np.float32)).T),
        "WkT": np.ascontiguousarray(np.asarray(Wk, np.float32).T),
        "WvT": np.ascontiguousarray(np.asarray(Wv, np.float32).T.astype(bf)),
        "WoT": np.ascontiguousarray(np.asarray(Wo, np.float32).T.astype(bf)),
        "bq8": np.ascontiguousarray(8.0 * np.asarray(bq, np.float32)),
        "bk": np.ascontiguousarray(np.asarray(bk, np.float32)),
        "bvb": np.ascontiguousarray(np.asarray(bv, np.float32).astype(bf)),
        "bob": np.ascontiguousarray(np.asarray(bo, np.float32).astype(bf)),
    }
    in_maps = []
    for i in range(B):
        m = dict(shared)
        m["XqT"] = np.ascontiguousarray(Q[i].T)
        m["XkT"] = np.ascontiguousarray(K[i].T)
        m["XvT"] = np.ascontiguousarray(V[i].T.astype(bf))
        qm = np.asarray(Q_mask[i], np.float32)
        qms = np.zeros((48, 512), np.float32)
        qms[0:16] = qm[0:512]
        qms[32:48] = qm[512:1024]
        m["Qms"] = qms.astype(bf)
        m["Km"] = np.ascontiguousarray(np.asarray(K_mask[i], np.float32))
        in_maps.append(m)
    res = run_bass_kernel_spmd(nc, in_maps, list(range(B)))
    return np.stack([res.results[i]["out"] for i in range(B)], axis=0)


# revision 30
# speedup vs baseline: 1.1587x; 1.0425x over previous
"""Trainium2 Bass kernel for nn_MultiHeadAttention (B=8, S=1024, HID=1024, NH=16).

Data-parallel over batch across 8 NeuronCores (1 batch element/core).
Host prep: weights pre-transposed (WqT pre-scaled by 8, bq by 8), activations
pre-transposed; Q/K path fp32r, V/O path bf16.

Per-core pipeline (single Tile program):
  1. qT/kT = WT @ XT as fp32r matmuls (1 cyc/row); bias added via ACT
     Identity (per-partition AP bias) during PSUM->SBUF copy; output f32r.
  2. v natural [seq,feat] via bf16 matmuls + K=1 bias row; masked by K_mask
     and augmented with [K_mask, 1] columns -> VM bf16.
  3. Attention, qc-half outer, head inner, ctx deferred one head (software
     pipeline): scores psum [128,1024] via 2 fp32r matmuls (K=64); DVE
     rowmax (negated); ACT exp(s-max) -> e bf16 SBUF; one DMA-xbar transpose
     per (h,qc) -> ET[kp, qb4, kb, q].
  4. ctx^T psum [66,512] = sum_kb VM' @ ET; rows 64/65 = U = sum(e*Km),
     D = sum(e); PSUM->SBUF copies on ACT.
  5. Per qc-half: scale = Q_mask/(U + S*1e-8*D) batched; ctx *= scale;
     out = ctx@WoT+bo for that half (overlaps other half's attention).
"""

import numpy as np
import ml_dtypes

import concourse.bass as bass
import concourse.tile as tile
from concourse import bacc, mybir
from concourse.bass_utils import run_bass_kernel_spmd

F32 = mybir.dt.float32
F32R = mybir.dt.float32r
BF16 = mybir.dt.bfloat16
AF = mybir.ActivationFunctionType
ALU = mybir.AluOpType
AX = mybir.AxisListType

B, S, HID, NH, DH = 8, 1024, 1024, 16, 64
P = 128
NP = HID // P        # 8 hid partition-blocks
NSB = S // P         # 8 seq blocks
NPAIR = NH // 2
CREN = float(S) * 1e-8

_CACHE = {}


def _build():
    nc = bacc.Bacc("TRN2", target_bir_lowering=False, debug=False, num_devices=B)

    XqT = nc.dram_tensor("XqT", [HID, S], F32R, kind="ExternalInput").ap()
    XkT = nc.dram_tensor("XkT", [HID, S], F32R, kind="ExternalInput").ap()
    XvT = nc.dram_tensor("XvT", [HID, S], BF16, kind="ExternalInput").ap()
    Qms = nc.dram_tensor("Qms", [48, 512], BF16, kind="ExternalInput").ap()
    Km = nc.dram_tensor("Km", [S], F32, kind="ExternalInput").ap()
    WqT = nc.dram_tensor("WqT", [HID, HID], F32R, kind="ExternalInput").ap()
    WkT = nc.dram_tensor("WkT", [HID, HID], F32R, kind="ExternalInput").ap()
    WvT = nc.dram_tensor("WvT", [HID, HID], BF16, kind="ExternalInput").ap()
    WoT = nc.dram_tensor("WoT", [HID, HID], BF16, kind="ExternalInput").ap()
    bq8 = nc.dram_tensor("bq8", [HID], F32, kind="ExternalInput").ap()
    bkv = nc.dram_tensor("bk", [HID], F32, kind="ExternalInput").ap()
    bvb = nc.dram_tensor("bvb", [HID], BF16, kind="ExternalInput").ap()
    bob = nc.dram_tensor("bob", [HID], BF16, kind="ExternalInput").ap()
    out = nc.dram_tensor("out", [S, HID], F32, kind="ExternalOutput").ap()
    scl_dram = nc.dram_tensor("scl_scratch", [32, 512], F32).ap()

    with tile.TileContext(nc) as tc:
        with (
            tc.tile_pool(name="consts", bufs=1) as consts,
            tc.tile_pool(name="xh", bufs=2) as xh,          # X^T halves
            tc.tile_pool(name="wch", bufs=2) as wch,        # Wq/Wk^T ob-chunks
            tc.tile_pool(name="wbf", bufs=1) as wbf,        # WvT -> WoT bf16
            tc.tile_pool(name="qk", bufs=1) as qkp,
            tc.tile_pool(name="vm", bufs=1) as vmp,
            tc.tile_pool(name="et", bufs=3) as etp,
            tc.tile_pool(name="epool", bufs=2) as epool,
            tc.tile_pool(name="ctx", bufs=1) as ctxp,
            tc.tile_pool(name="smalls", bufs=4) as smalls,
            tc.tile_pool(name="ud", bufs=1) as udp,
            tc.tile_pool(name="pf512", bufs=2) as pf512,
            tc.tile_pool(name="ps_big", bufs=3, space="PSUM") as ps_big,
            tc.tile_pool(name="ps_mid", bufs=2, space="PSUM") as ps_mid,
        ):
            # --- prefetch first projection operands before anything else ---
            def load_xhalf(x_dram, dtype, sc):
                xt = xh.tile([P, NP, 512], dtype, tag="xh")
                nc.sync.dma_start(
                    xt[:],
                    x_dram.rearrange("(m p) s -> p m s", p=P)
                    [:, :, sc * 512:(sc + 1) * 512])
                return xt

            def load_wchunk(w_dram, ob):
                wc = wch.tile([P, NP, P], F32R, tag="wch")
                nc.sync.dma_start(
                    wc[:],
                    w_dram.rearrange("(m p) f -> p m f", p=P)
                    [:, :, ob * P:(ob + 1) * P])
                return wc

            xq_h0 = load_xhalf(XqT, F32R, 0)
            wq0 = load_wchunk(WqT, 0)
            xq_halves = [xq_h0, load_xhalf(XqT, F32R, 1)]

            # ---------------- constants (on ACT DMA queue) ----------------
            km_pi = consts.tile([P, NSB], F32, name="km_pi")
            nc.gpsimd.dma_start(km_pi[:], Km.rearrange("(o p) -> p o", p=P))
            bq8p = consts.tile([P, NP], F32, name="bq8p")
            nc.gpsimd.dma_start(bq8p[:], bq8.rearrange("(o p) -> p o", p=P))
            bkp = consts.tile([P, NP], F32, name="bkp")
            nc.gpsimd.dma_start(bkp[:], bkv.rearrange("(o p) -> p o", p=P))
            onesb = consts.tile([1, P], BF16, name="onesb")
            nc.vector.memset(onesb[:], 1.0)
            brow = consts.tile([1, HID], BF16, name="brow")
            nc.gpsimd.dma_start(brow[:], bvb[None, :])
            ST = consts.tile([48, S], F32, name="ST")
            qmst = consts.tile([48, 512], BF16, name="qmst")
            nc.gpsimd.dma_start(qmst[:], Qms)

            # ---------------- q/k projections (fp32r) ----------------
            qt = qkp.tile([P, NP, S], F32R, name="qt")
            kt = qkp.tile([P, NP, S], F32R, name="kt")

            def qk_projection(x_dram, w_dram, biasp, dst, xhalves, w0):
                if xhalves is None:
                    xhalves = [load_xhalf(x_dram, F32R, sc) for sc in range(2)]
                for ob in range(NP):
                    wc = w0 if (ob == 0 and w0 is not None) \
                        else load_wchunk(w_dram, ob)
                    for sc in range(2):
                        pool = ps_mid if sc == 0 else ps_big
                        pp = pool.tile([P, 512], F32,
                                       tag="mid" if sc == 0 else "big")
                        for m in range(NP):
                            nc.tensor.matmul(pp[:], wc[:, m, :],
                                             xhalves[sc][:, m, :],
                                             start=(m == 0), stop=(m == NP - 1))
                        nc.scalar.activation(
                            dst[:, ob, sc * 512:(sc + 1) * 512], pp[:],
                            AF.Identity, bias=biasp[:, ob:ob + 1], scale=1.0)

            qk_projection(XqT, WqT, bq8p, qt, xq_halves, wq0)
            qk_projection(XkT, WkT, bkp, kt, None, None)

            # ---------------- v projection (bf16, natural layout) ----------
            VM = vmp.tile([P, NSB, NH, 66], BF16, name="VM")
            wvt = wbf.tile([P, NP, HID], BF16, tag="wbf")
            nc.sync.dma_start(wvt[:], WvT.rearrange("(m p) f -> p m f", p=P))
            for sc in range(2):
                xvt = xh.tile([P, NP, 512], BF16, tag="xh")
                nc.sync.dma_start(
                    xvt[:],
                    XvT.rearrange("(m p) s -> p m s", p=P)
                    [:, :, sc * 512:(sc + 1) * 512])
                for sb2 in range(4):
                    sb = sc * 4 + sb2
                    for oc in range(2):
                        pool = ps_mid if oc == 0 else ps_big
                        pp = pool.tile([P, 512], F32,
                                       tag="mid" if oc == 0 else "big")
                        for m in range(NP):
                            nc.tensor.matmul(
                                pp[:], xvt[:, m, sb2 * P:(sb2 + 1) * P],
                                wvt[:, m, oc * 512:(oc + 1) * 512],
                                start=(m == 0), stop=False)
                        nc.tensor.matmul(pp[:], onesb[:],
                                         brow[:, oc * 512:(oc + 1) * 512],
                                         start=False, stop=True)
                        nc.vector.tensor_scalar_mul(
                            VM[:, sb, oc * 8:(oc + 1) * 8, 0:64],
                            pp[:].rearrange("p (h d) -> p h d", h=8),
                            km_pi[:, sb:sb + 1])
            for sb in range(NSB):
                nc.vector.tensor_copy(
                    VM[:, sb, :, 64:65],
                    km_pi[:, sb:sb + 1, None].to_broadcast([P, NH, 1]))
            nc.vector.memset(VM[:, :, :, 65:66], 1.0)

            # ---------------- attention (software-pipelined) ----------------
            ctxu = ctxp.tile([P, NPAIR, S], BF16, name="ctxu")

            def emit_ctx_mm(h, qc, et_c):
                cp = ps_mid.tile([P, 512], F32, tag="mid")
                for kb in range(NSB):
                    nc.tensor.matmul(cp[0:66, :], VM[:, kb, h, :],
                                     et_c[:, :, kb, :],
                                     start=(kb == 0), stop=(kb == NSB - 1))
                return cp

            def emit_ctx_copy(h, qc, cp):
                j, pb = h // 2, 64 * (h % 2)
                idx = qc * 32 + h
                ud = udp.tile([2, 512], F32, tag="ud")
                nc.scalar.activation(ud[:], cp[64:66, :], AF.Copy)
                nc.sync.dma_start(ST[idx:idx + 1, :], ud[:])
                if pb == 0:
                    nc.vector.tensor_copy(
                        ctxu[0:64, j, qc * 512:(qc + 1) * 512], cp[0:64, :])
                else:
                    sg = pf512.tile([64, 512], BF16, tag="pf512")
                    nc.scalar.activation(sg[:], cp[0:64, :], AF.Copy)
                    nc.sync.dma_start(
                        ctxu[64:128, j, qc * 512:(qc + 1) * 512], sg[:])

            wot = None
            pending = []
            copyq = []
            for qc in range(2):
                for h in range(NH):
                    j, pb = h // 2, 64 * (h % 2)
                    # et_c[kp, qb4, kb, q] = e[qb4-block q, kb*128+kp]
                    et_c = etp.tile([P, 4, NSB, P], BF16, tag="etc")
                    hh = qc * 16 + h
                    if hh % 2 == 0:
                        e_buf = epool.tile([P, 4, S], BF16, tag="e")
                    else:
                        e_buf = xh.tile([P, 4, S], BF16, tag="xh")

                    def score_tile(qb4):
                        qb = qc * 4 + qb4
                        sp = ps_big.tile([P, S], F32, tag="big")
                        qs = slice(qb * P, (qb + 1) * P)
                        for kc in range(2):
                            ks = slice(kc * 512, (kc + 1) * 512)
                            nc.tensor.matmul(sp[:, ks], qt[pb:pb + 64, j, qs],
                                             kt[pb:pb + 64, j, ks],
                                             start=True, stop=True)
                        nmax = smalls.tile([P, 1], F32, tag="nmax")
                        nc.vector.tensor_reduce(nmax[:], sp[:], axis=AX.X,
                                                op=ALU.max, negate=True)
                        nc.scalar.activation(e_buf[:, qb4, :], sp[:],
                                             AF.Exp, bias=nmax[:], scale=1.0)

                    score_tile(0)
                    score_tile(1)
                    if len(pending) == 2:
                        ph, pqc, pet = pending.pop(0)
                        cp = emit_ctx_mm(ph, pqc, pet)
                        copyq.append((ph, pqc, cp))
                    score_tile(2)
                    score_tile(3)
                    nc.sync.dma_start(et_c[:], e_buf[:], transpose=True)
                    if len(copyq) == 2:
                        emit_ctx_copy(*copyq.pop(0))
                    pending.append((h, qc, et_c))
                for args in pending:
                    ph, pqc, pet = args
                    cp = emit_ctx_mm(ph, pqc, pet)
                    copyq.append((ph, pqc, cp))
                pending = []
                for args in copyq:
                    emit_ctx_copy(*args)
                copyq = []

                # ---- renorm + output projection for this qc-half ----
                r0 = qc * 32
                r0d = qc * 16
                scl = ST[r0:r0 + 16, 0:512]
                nc.vector.scalar_tensor_tensor(
                    scl, ST[r0:r0 + 16, 512:1024], CREN,
                    scl, ALU.mult, ALU.add)
                nc.vector.reciprocal(scl, scl)
                nc.vector.tensor_tensor(scl, scl, qmst[r0:r0 + 16, :],
                                        ALU.mult)
                nc.sync.dma_start(scl_dram[r0d:r0d + 16, :], scl)

                for j in range(NPAIR):
                    sd = pf512.tile([P, 512], F32, tag="pf512")
                    ia = r0d + 2 * j
                    nc.sync.dma_start(
                        sd[0:64, :],
                        scl_dram[ia:ia + 1, :].to_broadcast([64, 512]))
                    nc.sync.dma_start(
                        sd[64:128, :],
                        scl_dram[ia + 1:ia + 2, :].to_broadcast([64, 512]))
                    nc.gpsimd.tensor_tensor(
                        ctxu[:, j, qc * 512:(qc + 1) * 512],
                        ctxu[:, j, qc * 512:(qc + 1) * 512], sd[:], ALU.mult)

                if wot is None:
                    wot = wbf.tile([P, NP, HID], BF16, tag="wbf")
                    nc.sync.dma_start(wot[:],
                                      WoT.rearrange("(m p) f -> p m f", p=P))
                    nc.gpsimd.dma_start(brow[:], bob[None, :])
                for qb in range(qc * 4, qc * 4 + 4):
                    for oc in range(2):
                        op_ = ps_mid.tile([P, 512], F32, tag="mid")
                        for j in range(NPAIR):
                            nc.tensor.matmul(
                                op_[:], ctxu[:, j, qb * P:(qb + 1) * P],
                                wot[:, j, oc * 512:(oc + 1) * 512],
                                start=(j == 0), stop=False)
                        nc.tensor.matmul(op_[:], onesb[:],
                                         brow[:, oc * 512:(oc + 1) * 512],
                                         start=False, stop=True)
                        ot = pf512.tile([P, 512], F32, tag="pf512")
                        nc.vector.tensor_copy(ot[:], op_[:])
                        nc.sync.dma_start(
                            out[qb * P:(qb + 1) * P, oc * 512:(oc + 1) * 512],
                            ot[:])

    nc.compile()
    return nc


def kernel(Q, K, V, Q_mask, K_mask, Wq, bq, Wk, bk, Wv, bv, Wo, bo):
    if "nc" not in _CACHE:
        _CACHE["nc"] = _build()
    nc = _CACHE["nc"]
    Q = np.asarray(Q, np.float32)
    K = np.asarray(K, np.float32)
    V = np.asarray(V, np.float32)
    bf = ml_dtypes.bfloat16
    shared = {
        "WqT": np.ascontiguousarray((8.0 * np.asarray(Wq, np.float32)).T),
        "WkT": np.ascontiguousarray(np.asarray(Wk, np.float32).T),
        "WvT": np.ascontiguousarray(np.asarray(Wv, np.float32).T.astype(bf)),
        "WoT": np.ascontiguousarray(np.asarray(Wo, np.float32).T.astype(bf)),
        "bq8": np.ascontiguousarray(8.0 * np.asarray(bq, np.float32)),
        "bk": np.ascontiguousarray(np.asarray(bk, np.float32)),
        "bvb": np.ascontiguousarray(np.asarray(bv, np.float32).astype(bf)),
        "bob": np.ascontiguousarray(np.asarray(bo, np.float32).astype(bf)),
    }
    in_maps = []
    for i in range(B):
        m = dict(shared)
        m["XqT"] = np.ascontiguousarray(Q[i].T)
        m["XkT"] = np.ascontiguousarray(K[i].T)
        m["XvT"] = np.ascontiguousarray(V[i].T.astype(bf))
        qm = np.asarray(Q_mask[i], np.float32)
        qms = np.zeros((48, 512), np.float32)
        qms[0:16] = qm[0:512]
        qms[32:48] = qm[512:1024]
        m["Qms"] = qms.astype(bf)
        m["Km"] = np.ascontiguousarray(np.asarray(K_mask[i], np.float32))
        in_maps.append(m)
    res = run_bass_kernel_spmd(nc, in_maps, list(range(B)))
    return np.stack([res.results[i]["out"] for i in range(B)], axis=0)


# revision 31
# speedup vs baseline: 1.1615x; 1.0024x over previous
"""Trainium2 Bass kernel for nn_MultiHeadAttention (B=8, S=1024, HID=1024, NH=16).

Data-parallel over batch across 8 NeuronCores (1 batch element/core).
Host prep: weights pre-transposed (WqT pre-scaled by 8, bq by 8), activations
pre-transposed; Q/K path fp32r, V/O path bf16.

Per-core pipeline (single Tile program):
  1. qT/kT = WT @ XT as fp32r matmuls (1 cyc/row); bias added via ACT
     Identity (per-partition AP bias) during PSUM->SBUF copy; output f32r.
  2. v natural [seq,feat] via bf16 matmuls + K=1 bias row; masked by K_mask
     and augmented with [K_mask, 1] columns -> VM bf16.
  3. Attention, qc-half outer, head inner, ctx deferred one head (software
     pipeline): scores psum [128,1024] via 2 fp32r matmuls (K=64); DVE
     rowmax (negated); ACT exp(s-max) -> e bf16 SBUF; one DMA-xbar transpose
     per (h,qc) -> ET[kp, qb4, kb, q].
  4. ctx^T psum [66,512] = sum_kb VM' @ ET; rows 64/65 = U = sum(e*Km),
     D = sum(e); PSUM->SBUF copies on ACT.
  5. Per qc-half: scale = Q_mask/(U + S*1e-8*D) batched; ctx *= scale;
     out = ctx@WoT+bo for that half (overlaps other half's attention).
"""

import numpy as np
import ml_dtypes

import concourse.bass as bass
import concourse.tile as tile
from concourse import bacc, mybir
from concourse.bass_utils import run_bass_kernel_spmd

F32 = mybir.dt.float32
F32R = mybir.dt.float32r
BF16 = mybir.dt.bfloat16
AF = mybir.ActivationFunctionType
ALU = mybir.AluOpType
AX = mybir.AxisListType

B, S, HID, NH, DH = 8, 1024, 1024, 16, 64
P = 128
NP = HID // P        # 8 hid partition-blocks
NSB = S // P         # 8 seq blocks
NPAIR = NH // 2
CREN = float(S) * 1e-8

_CACHE = {}


def _build():
    nc = bacc.Bacc("TRN2", target_bir_lowering=False, debug=False, num_devices=B)

    XqT = nc.dram_tensor("XqT", [HID, S], F32R, kind="ExternalInput").ap()
    XkT = nc.dram_tensor("XkT", [HID, S], F32R, kind="ExternalInput").ap()
    XvT = nc.dram_tensor("XvT", [HID, S], BF16, kind="ExternalInput").ap()
    Qms = nc.dram_tensor("Qms", [48, 512], BF16, kind="ExternalInput").ap()
    Km = nc.dram_tensor("Km", [S], F32, kind="ExternalInput").ap()
    WqT = nc.dram_tensor("WqT", [HID, HID], F32R, kind="ExternalInput").ap()
    WkT = nc.dram_tensor("WkT", [HID, HID], F32R, kind="ExternalInput").ap()
    WvT = nc.dram_tensor("WvT", [HID, HID], BF16, kind="ExternalInput").ap()
    WoT = nc.dram_tensor("WoT", [HID, HID], BF16, kind="ExternalInput").ap()
    bq8 = nc.dram_tensor("bq8", [HID], F32, kind="ExternalInput").ap()
    bkv = nc.dram_tensor("bk", [HID], F32, kind="ExternalInput").ap()
    bvb = nc.dram_tensor("bvb", [HID], BF16, kind="ExternalInput").ap()
    bob = nc.dram_tensor("bob", [HID], BF16, kind="ExternalInput").ap()
    out = nc.dram_tensor("out", [S, HID], F32, kind="ExternalOutput").ap()
    scl_dram = nc.dram_tensor("scl_scratch", [32, 512], F32).ap()

    with tile.TileContext(nc) as tc:
        with (
            tc.tile_pool(name="consts", bufs=1) as consts,
            tc.tile_pool(name="xh", bufs=2) as xh,          # X^T halves
            tc.tile_pool(name="wch", bufs=2) as wch,        # Wq/Wk^T ob-chunks
            tc.tile_pool(name="wbf", bufs=1) as wbf,        # WvT -> WoT bf16
            tc.tile_pool(name="qk", bufs=1) as qkp,
            tc.tile_pool(name="vm", bufs=1) as vmp,
            tc.tile_pool(name="et", bufs=3) as etp,
            tc.tile_pool(name="epool", bufs=2) as epool,
            tc.tile_pool(name="ctx", bufs=1) as ctxp,
            tc.tile_pool(name="smalls", bufs=4) as smalls,
            tc.tile_pool(name="ud", bufs=1) as udp,
            tc.tile_pool(name="pf512", bufs=2) as pf512,
            tc.tile_pool(name="ps_big", bufs=3, space="PSUM") as ps_big,
            tc.tile_pool(name="ps_mid", bufs=2, space="PSUM") as ps_mid,
        ):
            # --- prefetch first projection operands before anything else ---
            def load_xhalf(x_dram, dtype, sc):
                xt = xh.tile([P, NP, 512], dtype, tag="xh")
                nc.sync.dma_start(
                    xt[:],
                    x_dram.rearrange("(m p) s -> p m s", p=P)
                    [:, :, sc * 512:(sc + 1) * 512])
                return xt

            def load_wchunk(w_dram, ob):
                wc = wch.tile([P, NP, P], F32R, tag="wch")
                nc.sync.dma_start(
                    wc[:],
                    w_dram.rearrange("(m p) f -> p m f", p=P)
                    [:, :, ob * P:(ob + 1) * P])
                return wc

            xq_h0 = load_xhalf(XqT, F32R, 0)
            wq0 = load_wchunk(WqT, 0)
            xq_halves = [xq_h0, load_xhalf(XqT, F32R, 1)]

            # ---------------- constants (on ACT DMA queue) ----------------
            km_pi = consts.tile([P, NSB], F32, name="km_pi")
            nc.gpsimd.dma_start(km_pi[:], Km.rearrange("(o p) -> p o", p=P))
            bq8p = consts.tile([P, NP], F32, name="bq8p")
            nc.gpsimd.dma_start(bq8p[:], bq8.rearrange("(o p) -> p o", p=P))
            bkp = consts.tile([P, NP], F32, name="bkp")
            nc.gpsimd.dma_start(bkp[:], bkv.rearrange("(o p) -> p o", p=P))
            onesb = consts.tile([1, P], BF16, name="onesb")
            nc.vector.memset(onesb[:], 1.0)
            brow = consts.tile([1, HID], BF16, name="brow")
            nc.gpsimd.dma_start(brow[:], bvb[None, :])
            ST = consts.tile([48, S], F32, name="ST")
            qmst = consts.tile([48, 512], BF16, name="qmst")
            nc.gpsimd.dma_start(qmst[:], Qms)

            # ---------------- q/k projections (fp32r) ----------------
            qt = qkp.tile([P, NP, S], F32R, name="qt")
            kt = qkp.tile([P, NP, S], F32R, name="kt")

            def qk_projection(x_dram, w_dram, biasp, dst, xhalves, w0):
                if xhalves is None:
                    xhalves = [load_xhalf(x_dram, F32R, sc) for sc in range(2)]
                for ob in range(NP):
                    wc = w0 if (ob == 0 and w0 is not None) \
                        else load_wchunk(w_dram, ob)
                    for sc in range(2):
                        pool = ps_mid if sc == 0 else ps_big
                        pp = pool.tile([P, 512], F32,
                                       tag="mid" if sc == 0 else "big")
                        for m in range(NP):
                            nc.tensor.matmul(pp[:], wc[:, m, :],
                                             xhalves[sc][:, m, :],
                                             start=(m == 0), stop=(m == NP - 1))
                        nc.scalar.activation(
                            dst[:, ob, sc * 512:(sc + 1) * 512], pp[:],
                            AF.Identity, bias=biasp[:, ob:ob + 1], scale=1.0)

            qk_projection(XqT, WqT, bq8p, qt, xq_halves, wq0)
            qk_projection(XkT, WkT, bkp, kt, None, None)

            # ---------------- v projection (bf16, natural layout) ----------
            VM = vmp.tile([P, NSB, NH, 66], BF16, name="VM")
            wvt = wbf.tile([P, NP, HID], BF16, tag="wbf")
            nc.sync.dma_start(wvt[:], WvT.rearrange("(m p) f -> p m f", p=P))
            for sc in range(2):
                xvt = xh.tile([P, NP, 512], BF16, tag="xh")
                nc.sync.dma_start(
                    xvt[:],
                    XvT.rearrange("(m p) s -> p m s", p=P)
                    [:, :, sc * 512:(sc + 1) * 512])
                for sb2 in range(4):
                    sb = sc * 4 + sb2
                    for oc in range(2):
                        pool = ps_mid if oc == 0 else ps_big
                        pp = pool.tile([P, 512], F32,
                                       tag="mid" if oc == 0 else "big")
                        for m in range(NP):
                            nc.tensor.matmul(
                                pp[:], xvt[:, m, sb2 * P:(sb2 + 1) * P],
                                wvt[:, m, oc * 512:(oc + 1) * 512],
                                start=(m == 0), stop=False)
                        nc.tensor.matmul(pp[:], onesb[:],
                                         brow[:, oc * 512:(oc + 1) * 512],
                                         start=False, stop=True)
                        nc.vector.tensor_scalar_mul(
                            VM[:, sb, oc * 8:(oc + 1) * 8, 0:64],
                            pp[:].rearrange("p (h d) -> p h d", h=8),
                            km_pi[:, sb:sb + 1])
            for sb in range(NSB):
                nc.vector.tensor_copy(
                    VM[:, sb, :, 64:65],
                    km_pi[:, sb:sb + 1, None].to_broadcast([P, NH, 1]))
            nc.vector.memset(VM[:, :, :, 65:66], 1.0)

            # ---------------- attention (software-pipelined) ----------------
            ctxu = ctxp.tile([P, NPAIR, S], BF16, name="ctxu")

            def emit_ctx_mm(h, qc, et_c):
                cp = ps_mid.tile([P, 512], F32, tag="mid")
                for kb in range(NSB):
                    nc.tensor.matmul(cp[0:66, :], VM[:, kb, h, :],
                                     et_c[:, :, kb, :],
                                     start=(kb == 0), stop=(kb == NSB - 1))
                return cp

            def emit_ctx_copy(h, qc, cp):
                j, pb = h // 2, 64 * (h % 2)
                idx = qc * 32 + h
                ud = udp.tile([2, 512], F32, tag="ud")
                nc.scalar.activation(ud[:], cp[64:66, :], AF.Copy)
                nc.sync.dma_start(ST[idx:idx + 1, :], ud[:])
                if pb == 0:
                    nc.vector.tensor_copy(
                        ctxu[0:64, j, qc * 512:(qc + 1) * 512], cp[0:64, :])
                else:
                    sg = pf512.tile([64, 512], BF16, tag="pf512")
                    nc.scalar.activation(sg[:], cp[0:64, :], AF.Copy)
                    nc.sync.dma_start(
                        ctxu[64:128, j, qc * 512:(qc + 1) * 512], sg[:])

            wot = None
            pending = []
            copyq = []
            for qc in range(2):
                for h in range(NH):
                    j, pb = h // 2, 64 * (h % 2)
                    # et_c[kp, qb4, kb, q] = e[qb4-block q, kb*128+kp]
                    et_c = etp.tile([P, 4, NSB, P], BF16, tag="etc")
                    hh = qc * 16 + h
                    if hh % 2 == 0:
                        e_buf = epool.tile([P, 4, S], BF16, tag="e")
                    else:
                        e_buf = xh.tile([P, 4, S], BF16, tag="xh")

                    def score_tile(qb4):
                        qb = qc * 4 + qb4
                        sp = ps_big.tile([P, S], F32, tag="big")
                        qs = slice(qb * P, (qb + 1) * P)
                        for kc in range(2):
                            ks = slice(kc * 512, (kc + 1) * 512)
                            nc.tensor.matmul(sp[:, ks], qt[pb:pb + 64, j, qs],
                                             kt[pb:pb + 64, j, ks],
                                             start=True, stop=True)
                        nmax = smalls.tile([P, 1], F32, tag="nmax")
                        nc.vector.tensor_reduce(nmax[:], sp[:], axis=AX.X,
                                                op=ALU.max, negate=True)
                        nc.scalar.activation(e_buf[:, qb4, :], sp[:],
                                             AF.Exp, bias=nmax[:], scale=1.0)

                    score_tile(0)
                    score_tile(1)
                    if len(pending) == 2:
                        ph, pqc, pet = pending.pop(0)
                        cp = emit_ctx_mm(ph, pqc, pet)
                        copyq.append((ph, pqc, cp))
                    score_tile(2)
                    score_tile(3)
                    nc.sync.dma_start(et_c[:], e_buf[:], transpose=True)
                    if len(copyq) == 2:
                        emit_ctx_copy(*copyq.pop(0))
                    pending.append((h, qc, et_c))
                for args in pending:
                    ph, pqc, pet = args
                    cp = emit_ctx_mm(ph, pqc, pet)
                    copyq.append((ph, pqc, cp))
                pending = []
                for args in copyq:
                    emit_ctx_copy(*args)
                copyq = []

                # ---- renorm + output projection for this qc-half ----
                r0 = qc * 32
                r0d = qc * 16
                scl = ST[r0:r0 + 16, 0:512]
                nc.vector.scalar_tensor_tensor(
                    scl, ST[r0:r0 + 16, 512:1024], CREN,
                    scl, ALU.mult, ALU.add)
                nc.vector.reciprocal(scl, scl)
                nc.vector.tensor_tensor(scl, scl, qmst[r0:r0 + 16, :],
                                        ALU.mult)
                nc.sync.dma_start(scl_dram[r0d:r0d + 16, :], scl)

                for j in range(NPAIR):
                    sd = pf512.tile([P, 512], F32, tag="pf512")
                    ia = r0d + 2 * j
                    nc.sync.dma_start(
                        sd[0:64, :],
                        scl_dram[ia:ia + 1, :].to_broadcast([64, 512]))
                    nc.sync.dma_start(
                        sd[64:128, :],
                        scl_dram[ia + 1:ia + 2, :].to_broadcast([64, 512]))
                    nc.vector.tensor_tensor(
                        ctxu[:, j, qc * 512:(qc + 1) * 512],
                        ctxu[:, j, qc * 512:(qc + 1) * 512], sd[:], ALU.mult)

                if wot is None:
                    wot = wbf.tile([P, NP, HID], BF16, tag="wbf")
                    nc.sync.dma_start(wot[:],
                                      WoT.rearrange("(m p) f -> p m f", p=P))
                    nc.gpsimd.dma_start(brow[:], bob[None, :])
                for qb in range(qc * 4, qc * 4 + 4):
                    for oc in range(2):
                        op_ = ps_mid.tile([P, 512], F32, tag="mid")
                        for j in range(NPAIR):
                            nc.tensor.matmul(
                                op_[:], ctxu[:, j, qb * P:(qb + 1) * P],
                                wot[:, j, oc * 512:(oc + 1) * 512],
                                start=(j == 0), stop=False)
                        nc.tensor.matmul(op_[:], onesb[:],
                                         brow[:, oc * 512:(oc + 1) * 512],
                                         start=False, stop=True)
                        ot = pf512.tile([P, 512], F32, tag="pf512")
                        nc.vector.tensor_copy(ot[:], op_[:])
                        nc.sync.dma_start(
                            out[qb * P:(qb + 1) * P, oc * 512:(oc + 1) * 512],
                            ot[:])

    nc.compile()
    return nc


def kernel(Q, K, V, Q_mask, K_mask, Wq, bq, Wk, bk, Wv, bv, Wo, bo):
    if "nc" not in _CACHE:
        _CACHE["nc"] = _build()
    nc = _CACHE["nc"]
    Q = np.asarray(Q, np.float32)
    K = np.asarray(K, np.float32)
    V = np.asarray(V, np.float32)
    bf = ml_dtypes.bfloat16
    shared = {
        "WqT": np.ascontiguousarray((8.0 * np.asarray(Wq, np.float32)).T),
        "WkT": np.ascontiguousarray(np.asarray(Wk, np.float32).T),
        "WvT": np.ascontiguousarray(np.asarray(Wv, np.float32).T.astype(bf)),
        "WoT": np.ascontiguousarray(np.asarray(Wo, np.float32).T.astype(bf)),
        "bq8": np.ascontiguousarray(8.0 * np.asarray(bq, np.float32)),
        "bk": np.ascontiguousarray(np.asarray(bk, np.float32)),
        "bvb": np.ascontiguousarray(np.asarray(bv, np.float32).astype(bf)),
        "bob": np.ascontiguousarray(np.asarray(bo, np.float32).astype(bf)),
    }
    in_maps = []
    for i in range(B):
        m = dict(shared)
        m["XqT"] = np.ascontiguousarray(Q[i].T)
        m["XkT"] = np.ascontiguousarray(K[i].T)
        m["XvT"] = np.ascontiguousarray(V[i].T.astype(bf))
        qm = np.asarray(Q_mask[i], np.float32)
        qms = np.zeros((48, 512), np.float32)
        qms[0:16] = qm[0:512]
        qms[32:48] = qm[512:1024]
        m["Qms"] = qms.astype(bf)
        m["Km"] = np.ascontiguousarray(np.asarray(K_mask[i], np.float32))
        in_maps.append(m)
    res = run_bass_kernel_spmd(nc, in_maps, list(range(B)))
    return np.stack([res.results[i]["out"] for i in range(B)], axis=0)
